# revision 14
# baseline (speedup 1.0000x reference)
"""Single-head causal attention (V=K source bug) on 8 trn2 NeuronCores.

Problem: x[4,2048,1024], W_Q/W_K/W_V[64,1024] (W_V unused by reference).
  Q = x @ W_Q.T ; K = x @ W_K.T ; V = K (reference bug)
  out = softmax(mask(Q K^T / sqrt(1024))) @ V      -> [4,2048,64]

Sharding: 2 cores per batch (core i: batch = i % 4, role r = i // 4).
Each batch's 8 query tiles of 256 rows split by parity (r=0 even, r=1 odd).
ONE SPMD graph for all 8 cores; per-core differences folded into data:
x^T is sent column-permuted (own tiles first), per-core 0/1 `scal` flags
resolve the other-side boundary tiles. Causal diag masks are built
ON-DEVICE with affine_select (they are slot- and core-invariant).

v2 pipeline (per core):
 * x^T sent slab-major [4, 128, 8, 512]: each slab DMA is 128 descriptors
   of 8KB contiguous runs; DMAs are emitted first so doorbells ring as
   soon as the preamble barrier clears. Slab order s0..s3.
 * Fused KQ projection: lhsT = [W_K | W_Q] [128,128] -> one M=128 pass
   per slab gives K (psum rows 0-63) and Q (rows 64-127) together.
   K copied same-partition into kstk[0:64]; Q staged then dup-DMA'd to
   partitions 0-63 (S matmuls contract parts 0-63 only).
 * Exp groups ordered by slab arrival; group (0,0) split into two
   2-chunk halves so the chain starts right after s0. ~11.5us serial
   ACT chain is the floor; everything else overlaps it.
 * PV matmuls interleaved group-by-group behind the exps (lhsT=[V|ones]
   accumulates the softmax denominator in psum row 64); only the last
   group's PV + copy + out-DMA trail the final exp.
 * Host divides by the denominator row and transposes back.
"""

import os
import sys

sys.path.insert(0, "/opt/trn_rl_repo")

import numpy as np
import ml_dtypes

BF16 = ml_dtypes.bfloat16

B, T, C, D = 4, 2048, 1024, 64
N_CORES = 8
QTILE = 256          # query rows per slot
N_SLOTS = 4
CHUNK = 128          # key chunk
SCALE = C ** -0.5
N_WARMUP = 56        # HAM warmup matmuls (cover the DMA wait before proj0)

TRACE = False
TRACE_CORES = None
LAST_RESULTS = None


# groups: (slot j, [permuted chunks], kind)
#   kinds: 'own_diag' (slices 2-3 diag MUL), 'own_diag01' (slices 0-1 MUL),
#          'oth_tail' (slices 2-3 scal), 'mixed' (0-1 MUL, 2-3 scal), 'plain'
# slab of permuted chunk c: c // 4. qT slot j lives in slab j // 2.
GROUPS_BY_SLAB = {
    0: [(0, [0, 1], "own_diag01"), (1, [0, 1, 2, 3], "own_diag")],
    1: [(2, [0, 1, 2, 3], "plain"), (3, [0, 1, 2, 3], "plain"),
        (3, [4, 5, 6, 7], "own_diag")],
    2: [(0, [8, 9], "oth01"), (1, [8, 9, 10, 11], "oth_tail"),
        (2, [8, 9, 10, 11], "plain"), (3, [8, 9, 10, 11], "plain")],
    3: [(2, [4, 5, 12, 13], "mixed"), (3, [12, 13, 14, 15], "oth_tail")],
}
# PV chunk counts per slot (for start/stop flags)
NCH_OF_SLOT = {0: 4, 1: 8, 2: 12, 3: 16}


def _build_graph():
    import concourse.bass as bass
    import concourse.mybir as mybir
    import concourse.tile as tile
    from concourse import bacc
    from concourse.masks import make_identity
    from contextlib import ExitStack

    fp32 = mybir.dt.float32
    bf16 = mybir.dt.bfloat16

    nc = bacc.Bacc(
        "TRN2",
        target_bir_lowering=False,
        debug=False,
        num_devices=N_CORES,
    )

    # x slab-major: [slab, part, cchunk, 512 cols]
    x4 = nc.dram_tensor("x4", [4, 128, C // CHUNK, 512], bf16,
                        kind="ExternalInput").ap()
    # [W_K | W_Q] per c-chunk: [part, cchunk, 128]
    wkq = nc.dram_tensor("wkq", [128, C // CHUNK, 2 * D], bf16,
                         kind="ExternalInput").ap()
    scald = nc.dram_tensor(
        "scal", [CHUNK, N_SLOTS], fp32, kind="ExternalInput"
    ).ap()
    out = nc.dram_tensor(
        "out", [D + 1, N_SLOTS * QTILE], fp32, kind="ExternalOutput"
    ).ap()

    CCH = C // CHUNK               # 8 contraction chunks
    NCH = T // CHUNK               # 16 key chunks

    with tile.TileContext(nc) as tc, ExitStack() as ctx:
        consts = ctx.enter_context(tc.tile_pool(name="consts", bufs=1))
        xpool = ctx.enter_context(tc.tile_pool(name="xpool", bufs=1))
        kqpool = ctx.enter_context(tc.tile_pool(name="kqpool", bufs=1))
        ptpool = ctx.enter_context(tc.tile_pool(name="ptpool", bufs=10))
        opool = ctx.enter_context(tc.tile_pool(name="opool", bufs=2))
        psP = ctx.enter_context(tc.tile_pool(name="psP", bufs=2, space="PSUM"))
        psS = ctx.enter_context(tc.tile_pool(name="psS", bufs=2, space="PSUM"))
        psO = ctx.enter_context(tc.tile_pool(name="psO", bufs=2, space="PSUM"))

        # ---- input DMAs first: doorbells ring right after the preamble ----
        scal_sb = consts.tile([128, N_SLOTS], fp32)
        nc.sync.dma_start(out=scal_sb, in_=scald)
        w_sb = consts.tile([128, CCH, 2 * D], bf16)
        nc.sync.dma_start(out=w_sb, in_=wkq)
        xs = []
        for s in range(4):
            xsl = xpool.tile([128, CCH, 512], bf16, name=f"xslab{s}")
            xs.append(xsl)
            nc.sync.dma_start(out=xsl, in_=x4[s])

        # ---- constants / warmup (overlap the DMA wait) ----
        warm_src = consts.tile([128, 128], bf16)
        nc.vector.memset(warm_src, 0.0)
        warm_ps = psP.tile([128, 128], fp32, tag="proj")
        for w in range(N_WARMUP):
            nc.tensor.matmul(
                warm_ps, lhsT=warm_src, rhs=warm_src,
                start=(w == 0), stop=(w == N_WARMUP - 1),
            )
        ident = consts.tile([128, 128], bf16)
        make_identity(nc, ident)
        warm = consts.tile([1, 1], fp32)
        nc.vector.memset(warm, 0.0)
        nc.scalar.activation(warm, warm, mybir.ActivationFunctionType.Exp)

        # causal diag masks, identical for every slot/core:
        # cols 0:256   valid iff p <= f      (h=0 chunk)
        # cols 256:512 valid iff p+128 <= f  (h=1 chunk)
        masks = consts.tile([128, 2 * QTILE], bf16)
        nc.gpsimd.memset(masks, 1.0)
        nc.gpsimd.affine_select(
            out=masks[:, 0:QTILE], in_=masks[:, 0:QTILE],
            compare_op=mybir.AluOpType.is_ge, fill=0.0,
            base=0, channel_multiplier=-1, pattern=[[1, QTILE]],
        )
        nc.gpsimd.affine_select(
            out=masks[:, QTILE:], in_=masks[:, QTILE:],
            compare_op=mybir.AluOpType.is_ge, fill=0.0,
            base=-CHUNK, channel_multiplier=-1, pattern=[[1, QTILE]],
        )

        # ---- persistent SBUF state ----
        kstk = kqpool.tile([64, NCH, CHUNK], bf16)       # K^T, parts 0-63
        qT = kqpool.tile([64, N_SLOTS * QTILE], bf16)    # Q^T, parts 0-63
        qstage = kqpool.tile([128, 2 * 512], bf16)       # Q staging (parts 64-127)
        vones = kqpool.tile([128, NCH, D + 1], bf16)     # V natural + ones col
        nc.vector.memset(vones[:, :, D:D + 1], 1.0)

        def filler(n, tag):
            f_ps = psP.tile([128, 128], fp32, tag="proj", name=f"warmf_{tag}")
            for w in range(n):
                nc.tensor.matmul(
                    f_ps, lhsT=warm_src, rhs=warm_src,
                    start=(w == 0), stop=(w == n - 1),
                )

        def proj(s):
            """Fused KQ projection of slab s: K -> kstk chunks 4s..4s+3,
            Q (slabs 0,1 only) -> qstage -> dup-DMA to qT."""
            p_ps = psP.tile([128, 512], fp32, tag="proj", name=f"proj{s}")
            for c in range(CCH):
                nc.tensor.matmul(
                    p_ps, lhsT=w_sb[:, c, :], rhs=xs[s][:, c, :],
                    start=(c == 0), stop=(c == CCH - 1),
                )
            # K rows 0-63, same-partition copy. s0 on scalar (ACT still
            # idle then); s1-s3 on vector so the exp chain never stalls.
            # (gpsimd cannot read PSUM.)
            ksl = kstk[:, 4 * s:4 * s + 4, :].rearrange("p c k -> p (c k)")
            if s == 0:
                nc.scalar.copy(ksl, p_ps[0:64, :])
            else:
                nc.vector.tensor_copy(ksl, p_ps[0:64, :])
            if s < 2:
                # Q rows 64-127: stage then cross-partition dup
                nc.vector.tensor_copy(
                    qstage[64:128, s * 512:(s + 1) * 512], p_ps[64:128, :]
                )
                nc.gpsimd.dma_start(
                    out=qT[:, s * 512:(s + 1) * 512],
                    in_=qstage[64:128, s * 512:(s + 1) * 512],
                )

        def transp(c0, n):
            """V natural (+ones) for chunks c0..c0+n-1 (pairs)."""
            for p0 in range(c0, c0 + n, 2):
                pt2 = psP.tile([128, 128], bf16, tag="proj", name=f"tp{p0}")
                for dk in range(2):
                    nc.tensor.transpose(
                        pt2[:, dk * 64:(dk + 1) * 64],
                        in_=kstk[:, p0 + dk, :],
                        identity=ident[0:64, 0:64],
                    )
                nc.vector.tensor_copy(vones[:, p0:p0 + 2, 0:D], pt2)

        pt_tiles = {}

        def sexp(j, gch, kind):
            """S^T matmuls + exp (+ masks) for one group of slot j."""
            n = len(gch)
            s_ps = psS.tile([128, n * QTILE], fp32, tag="s",
                            name=f"sps{j}_{gch[0]}")
            for sl, cc in enumerate(gch):
                nc.tensor.matmul(
                    s_ps[:, sl * QTILE:(sl + 1) * QTILE],
                    lhsT=kstk[:, cc, :],
                    rhs=qT[:, j * QTILE:(j + 1) * QTILE],
                    start=True, stop=True,
                )
            pt = ptpool.tile([128, n * QTILE], bf16, tag="pt",
                             name=f"pt{j}_{gch[0]}")
            nc.scalar.activation(
                pt, s_ps, mybir.ActivationFunctionType.Exp, scale=SCALE
            )
            if kind in ("own_diag01", "mixed"):
                nc.vector.tensor_mul(pt[:, 0:2 * QTILE], pt[:, 0:2 * QTILE],
                                     masks)
            if kind == "own_diag":
                nc.vector.tensor_mul(pt[:, 2 * QTILE:], pt[:, 2 * QTILE:],
                                     masks)
            if kind in ("oth_tail", "mixed"):
                nc.vector.tensor_scalar_mul(
                    pt[:, 2 * QTILE:], pt[:, 2 * QTILE:], scal_sb[:, j:j + 1]
                )
            if kind == "oth01":
                nc.vector.tensor_scalar_mul(pt, pt, scal_sb[:, j:j + 1])
            pt_tiles[(j, gch[0])] = pt

        # PSUM discipline: only ONE accumulation chain may be open per psum
        # bank at a time (an interleaved start=True wipes the open chain).
        # So each PV group is a CLOSED 4-matmul chain into a small rotating
        # psum tile; cross-group accumulation happens in SBUF on vector.
        o_acc = kqpool.tile([D + 1, N_SLOTS, QTILE], fp32)
        pv_done = {j: 0 for j in range(N_SLOTS)}

        def pv(j, gch):
            """PV for one group of slot j (closed psum chain + SBUF accum);
            DMA out when the slot completes."""
            pt = pt_tiles.pop((j, gch[0]))
            # full-bank tile (2KB/part): a start=True chain may clobber its
            # whole psum bank, so no two live tiles may share one.
            g_ps = psO.tile([D + 1, 2 * QTILE], fp32, tag="o",
                            name=f"ops{j}_{gch[0]}")[:, 0:QTILE]
            for sl, cc in enumerate(gch):
                nc.tensor.matmul(
                    g_ps,
                    lhsT=vones[:, cc, :],
                    rhs=pt[:, sl * QTILE:(sl + 1) * QTILE],
                    start=(sl == 0), stop=(sl == len(gch) - 1),
                )
            first = pv_done[j] == 0
            pv_done[j] += len(gch)
            if first:
                nc.vector.tensor_copy(o_acc[:, j, :], g_ps)
            else:
                nc.vector.tensor_add(o_acc[:, j, :], o_acc[:, j, :], g_ps)
            if pv_done[j] == NCH_OF_SLOT[j]:
                nc.gpsimd.dma_start(
                    out=out[:, j * QTILE:(j + 1) * QTILE], in_=o_acc[:, j, :]
                )

        # ---- emission order ~= execution order ----
        proj(0)
        for j, gch, kind in GROUPS_BY_SLAB[0]:
            sexp(j, gch, kind)
        transp(0, 4)
        filler(8, "s1")
        proj(1)
        g1 = GROUPS_BY_SLAB[1]
        sexp(*g1[0])
        pv(0, [0, 1])           # (0,0a)
        sexp(*g1[1])
        pv(1, [0, 1, 2, 3])     # (1,0)
        sexp(*g1[2])
        transp(4, 4)
        filler(8, "s2")
        proj(2)
        g2 = GROUPS_BY_SLAB[2]
        sexp(*g2[0])
        pv(2, [0, 1, 2, 3])     # (2,0)
        sexp(*g2[1])
        pv(3, [0, 1, 2, 3])     # (3,0)
        sexp(*g2[2])
        pv(3, [4, 5, 6, 7])     # (3,2)
        sexp(*g2[3])
        pv(0, [8, 9])           # (0,0b) -> finalizes slot 0
        transp(8, 4)
        filler(16, "s3")
        proj(3)
        g3 = GROUPS_BY_SLAB[3]
        pv(1, [8, 9, 10, 11])   # (1,1) -> finalizes slot 1
        sexp(*g3[0])
        pv(2, [8, 9, 10, 11])   # (2,1)
        transp(12, 4)
        sexp(*g3[1])
        pv(3, [8, 9, 10, 11])   # (3,1)
        pv(2, [4, 5, 12, 13])   # (2,2) -> finalizes slot 2
        pv(3, [12, 13, 14, 15])  # (3,3) -> finalizes slot 3

    nc.compile()
    return nc


_NC_CACHE = None


def _get_nc():
    global _NC_CACHE
    if _NC_CACHE is None:
        _NC_CACHE = _build_graph()
    return _NC_CACHE


def _perm_tiles(r):
    """permuted 256-col tile order: own tiles (2j+r) first, then others."""
    own = [2 * j + r for j in range(N_SLOTS)]
    oth = [2 * j + (1 - r) for j in range(N_SLOTS)]
    return own + oth


def _host_prep(x, W_Q, W_K):
    in_maps = []
    w = np.concatenate([W_K.T, W_Q.T], axis=1).astype(BF16)  # [1024, 128]
    wkq = np.ascontiguousarray(w.reshape(8, 128, 2 * D).transpose(1, 0, 2))
    for i in range(N_CORES):
        b, r = i % B, i // B
        perm = _perm_tiles(r)
        xt = x[b].T.astype(BF16)  # [1024, 2048]
        cols = np.concatenate(
            [np.arange(QTILE * p, QTILE * p + QTILE) for p in perm]
        )
        xkt = xt[:, cols]  # [1024, 2048] permuted
        # slab-major: [slab, part, cchunk, 512]
        x4 = np.ascontiguousarray(
            xkt.reshape(8, 128, 4, 512).transpose(2, 1, 0, 3)
        )
        sc = np.full((CHUNK, N_SLOTS), float(r), dtype=np.float32)
        in_maps.append({"x4": x4, "wkq": wkq, "scal": sc})
    return in_maps


def _ensure_ntff_hook():
    """Install the antenv.axon_hooks shim so trace=True works under axon."""
    import types

    try:
        from antenv.axon_hooks import get_axon_ntff_profile_hook  # noqa: F401

        return
    except ImportError:
        pass
    import antenv

    mod = types.ModuleType("antenv.axon_hooks")
    mod._hook = None

    def set_axon_ntff_profile_hook(h):
        mod._hook = h

    def get_axon_ntff_profile_hook():
        return mod._hook

    mod.set_axon_ntff_profile_hook = set_axon_ntff_profile_hook
    mod.get_axon_ntff_profile_hook = get_axon_ntff_profile_hook
    sys.modules["antenv.axon_hooks"] = mod
    antenv.axon_hooks = mod
    try:
        from trn_agent_boot.trn_boot import _ntff_profile_via_ctypes

        hook = _ntff_profile_via_ctypes("/opt/axon/libaxon_pjrt.so")
        if hook is not None:
            set_axon_ntff_profile_hook(hook)
    except Exception as e:  # degrade to no tracing
        print(f"ntff hook install failed: {e}")


def kernel(x, W_Q, W_K, W_V=None, **_unused):
    global LAST_RESULTS
    if TRACE:
        _ensure_ntff_hook()
    x = np.asarray(x, dtype=np.float32)
    W_Q = np.asarray(W_Q, dtype=np.float32)
    W_K = np.asarray(W_K, dtype=np.float32)

    from concourse.bass_utils import run_bass_kernel_spmd

    nc = _get_nc()
    in_maps = _host_prep(x, W_Q, W_K)
    res = run_bass_kernel_spmd(
        nc,
        in_maps,
        core_ids=list(range(N_CORES)),
        trace=TRACE,
        trace_cores=TRACE_CORES,
    )
    LAST_RESULTS = res

    y = np.empty((B, T, D), dtype=np.float32)
    for i in range(N_CORES):
        b, r = i % B, i // B
        ot = res.results[i]["out"]  # [65, 1024]
        o = ot[0:D, :] / ot[D:D + 1, :]
        for j in range(N_SLOTS):
            t0 = QTILE * (2 * j + r)
            y[b, t0:t0 + QTILE, :] = o[:, j * QTILE:(j + 1) * QTILE].T
    return y


# revision 27
# speedup vs baseline: 1.2157x; 1.2157x over previous
"""Single-head causal attention (V=K source bug) on 8 trn2 NeuronCores.

Problem: x[4,2048,1024], W_Q/W_K/W_V[64,1024] (W_V unused by reference).
  Q = x @ W_Q.T ; K = x @ W_K.T ; V = K (reference bug)
  out = softmax(mask(Q K^T / sqrt(1024))) @ V      -> [4,2048,64]

Sharding: 2 cores per batch (core i: batch = i % 4, role r = i // 4).
Each batch's 8 query tiles of 256 rows split by parity (r=0 even, r=1 odd).
ONE SPMD graph for all 8 cores; per-core differences folded into data:
x^T is sent column-permuted (own tiles first), per-core 0/1 `scal` flags
resolve the other-side boundary tiles. Causal diag masks are built
ON-DEVICE with affine_select (they are slot- and core-invariant).

v2 pipeline (per core):
 * x^T sent slab-major [4, 128, 8, 512]: each slab DMA is 128 descriptors
   of 8KB contiguous runs; DMAs are emitted first so doorbells ring as
   soon as the preamble barrier clears. Slab order s0..s3.
 * Fused KQ projection: lhsT = [W_K | W_Q] [128,128] -> one M=128 pass
   per slab gives K (psum rows 0-63) and Q (rows 64-127) together.
   K copied same-partition into kstk[0:64]; Q staged then dup-DMA'd to
   partitions 0-63 (S matmuls contract parts 0-63 only).
 * Exp groups ordered by slab arrival; group (0,0) split into two
   2-chunk halves so the chain starts right after s0. ~11.5us serial
   ACT chain is the floor; everything else overlaps it.
 * PV matmuls interleaved group-by-group behind the exps (lhsT=[V|ones]
   accumulates the softmax denominator in psum row 64); only the last
   group's PV + copy + out-DMA trail the final exp.
 * Host divides by the denominator row and transposes back.
"""

import os
import sys

sys.path.insert(0, "/opt/trn_rl_repo")

import numpy as np
import ml_dtypes

BF16 = ml_dtypes.bfloat16

B, T, C, D = 4, 2048, 1024, 64
N_CORES = 8
QTILE = 256          # query rows per slot
N_SLOTS = 4
CHUNK = 128          # key chunk
SCALE = C ** -0.5
N_WARMUP = 96        # gap-filler matmul budget (keeps PE p-state up)

TRACE = False
TRACE_CORES = None
LAST_RESULTS = None


# groups: (slot j, [permuted chunks], kind)
#   kinds: 'own_diag' (slices 2-3 diag MUL), 'own_diag01' (slices 0-1 MUL),
#          'oth_tail' (slices 2-3 scal), 'mixed' (0-1 MUL, 2-3 scal), 'plain'
# slab of permuted chunk c: c // 4. qT slot j lives in slab j // 2.
GROUPS_BY_SLAB = {
    0: [(0, [0, 1], "own_diag01"), (1, [0, 1, 2, 3], "own_diag")],
    1: [(2, [0, 1, 2, 3], "plain"), (3, [0, 1, 2, 3], "plain"),
        (3, [4, 5, 6, 7], "own_diag")],
    2: [(0, [8, 9], "oth01"), (1, [8, 9, 10, 11], "oth_tail"),
        (2, [8, 9, 10, 11], "plain"), (3, [8, 9, 10, 11], "plain")],
    3: [(2, [4, 5, 12, 13], "mixed"), (3, [12, 13, 14, 15], "oth_tail")],
}
# PV chunk counts per slot (for start/stop flags)
NCH_OF_SLOT = {0: 4, 1: 8, 2: 12, 3: 16}


def _build_graph():
    import concourse.bass as bass
    import concourse.mybir as mybir
    import concourse.tile as tile
    from concourse.tile import add_dep_helper
    from concourse import bacc
    from concourse.masks import make_identity
    from contextlib import ExitStack

    fp32 = mybir.dt.float32
    bf16 = mybir.dt.bfloat16

    nc = bacc.Bacc(
        "TRN2",
        target_bir_lowering=False,
        debug=False,
        num_devices=N_CORES,
    )

    # x slab-major: [slab, part, cchunk, 512 cols]
    x4 = nc.dram_tensor("x4", [4, 128, C // CHUNK, 512], bf16,
                        kind="ExternalInput").ap()
    # [W_K | W_Q] per c-chunk: [part, cchunk, 128]
    wkq = nc.dram_tensor("wkq", [128, C // CHUNK, 2 * D], bf16,
                         kind="ExternalInput").ap()
    scald = nc.dram_tensor(
        "scal", [CHUNK, N_SLOTS], fp32, kind="ExternalInput"
    ).ap()
    out = nc.dram_tensor(
        "out", [D + 1, N_SLOTS * QTILE], fp32, kind="ExternalOutput"
    ).ap()

    CCH = C // CHUNK               # 8 contraction chunks
    NCH = T // CHUNK               # 16 key chunks

    with tile.TileContext(nc) as tc, ExitStack() as ctx:
        consts = ctx.enter_context(tc.tile_pool(name="consts", bufs=1))
        xpool = ctx.enter_context(tc.tile_pool(name="xpool", bufs=1))
        kqpool = ctx.enter_context(tc.tile_pool(name="kqpool", bufs=1))
        ptpool = ctx.enter_context(tc.tile_pool(name="ptpool", bufs=10))
        opool = ctx.enter_context(tc.tile_pool(name="opool", bufs=2))
        psP = ctx.enter_context(tc.tile_pool(name="psP", bufs=2, space="PSUM"))
        psS = ctx.enter_context(tc.tile_pool(name="psS", bufs=2, space="PSUM"))
        psO = ctx.enter_context(tc.tile_pool(name="psO", bufs=2, space="PSUM"))

        # ---- input DMAs first on the sync queue (strict arrival order:
        # wkq, then slabs s0..s3); scal rides the idle gpsimd queue ----
        w_sb = consts.tile([128, CCH, 2 * D], bf16)
        nc.sync.dma_start(out=w_sb, in_=wkq)
        xs = []
        for s in range(4):
            xsl = xpool.tile([128, CCH, 512], bf16, name=f"xslab{s}")
            xs.append(xsl)
            nc.sync.dma_start(out=xsl, in_=x4[s])
        scal_sb = consts.tile([128, N_SLOTS], fp32)
        nc.gpsimd.dma_start(out=scal_sb, in_=scald)

        # ---- constants ----
        warm_src = consts.tile([128, 128], bf16)
        nc.vector.memset(warm_src, 0.0)
        ident = consts.tile([128, 128], bf16)
        make_identity(nc, ident)

        # causal diag masks, identical for every slot/core:
        # cols 0:256   valid iff p <= f      (h=0 chunk)
        # cols 256:512 valid iff p+128 <= f  (h=1 chunk)
        masks = consts.tile([128, 2 * QTILE], bf16)
        nc.gpsimd.memset(masks, 1.0)
        nc.gpsimd.affine_select(
            out=masks[:, 0:QTILE], in_=masks[:, 0:QTILE],
            compare_op=mybir.AluOpType.is_ge, fill=0.0,
            base=0, channel_multiplier=-1, pattern=[[1, QTILE]],
        )
        nc.gpsimd.affine_select(
            out=masks[:, QTILE:], in_=masks[:, QTILE:],
            compare_op=mybir.AluOpType.is_ge, fill=0.0,
            base=-CHUNK, channel_multiplier=-1, pattern=[[1, QTILE]],
        )

        # ---- persistent SBUF state ----
        kstk = kqpool.tile([64, NCH * CHUNK], bf16)      # K^T, parts 0-63
        qT = kqpool.tile([64, N_SLOTS * QTILE], bf16)    # Q^T, parts 0-63
        qstage = kqpool.tile([128, 2 * 512], bf16)       # Q staging (parts 64-127)
        vones = kqpool.tile([128, NCH, D + 1], bf16)     # V natural + ones col
        nc.vector.memset(vones[:, :, D:D + 1], 1.0)

        def filler(n, tag):
            f_ps = psP.tile([128, 128], fp32, tag="proj", name=f"warmf_{tag}")
            for w in range(n):
                nc.tensor.matmul(
                    f_ps, lhsT=warm_src, rhs=warm_src,
                    start=(w == 0), stop=(w == n - 1),
                )

        def proj(s):
            """Fused KQ projection of slab s: K -> kstk chunks 4s..4s+3,
            Q (slabs 0,1 only) -> qstage -> dup-DMA to qT."""
            p_ps = psP.tile([128, 512], fp32, tag="proj", name=f"proj{s}")
            for c in range(CCH):
                nc.tensor.matmul(
                    p_ps, lhsT=w_sb[:, c, :], rhs=xs[s][:, c, :],
                    start=(c == 0), stop=(c == CCH - 1),
                )
            # K rows 0-63, same-partition copy. s0 on scalar (ACT still
            # idle then); s1-s3 on vector so the exp chain never stalls.
            # (gpsimd cannot read PSUM.)
            ksl = kstk[:, 512 * s:512 * (s + 1)]
            if s == 0:
                nc.scalar.copy(ksl, p_ps[0:64, :])
            else:
                nc.vector.tensor_copy(ksl, p_ps[0:64, :])
            if s < 2:
                # Q rows 64-127: stage then cross-partition dup
                nc.vector.tensor_copy(
                    qstage[64:128, s * 512:(s + 1) * 512], p_ps[64:128, :]
                )
                nc.gpsimd.dma_start(
                    out=qT[:, s * 512:(s + 1) * 512],
                    in_=qstage[64:128, s * 512:(s + 1) * 512],
                )

        vcopy_inst = {}

        def transp(c0, n):
            """V natural (+ones) for chunks c0..c0+n-1 (pairs)."""
            for p0 in range(c0, c0 + n, 2):
                pt2 = psP.tile([128, 128], bf16, tag="proj", name=f"tp{p0}")
                for dk in range(2):
                    nc.tensor.transpose(
                        pt2[:, dk * 64:(dk + 1) * 64],
                        in_=kstk[:, (p0 + dk) * CHUNK:(p0 + dk + 1) * CHUNK],
                        identity=ident[0:64, 0:64],
                    )
                ci = nc.vector.tensor_copy(vones[:, p0:p0 + 2, 0:D], pt2)
                vcopy_inst[p0] = vcopy_inst[p0 + 1] = ci

        pt_tiles = {}

        def sexp(j, gch, kind):
            """S^T matmuls + exp (+ masks) for one group of slot j."""
            n = len(gch)
            s_ps = psS.tile([128, n * QTILE], fp32, tag="s",
                            name=f"sps{j}_{gch[0]}")
            for sl, cc in enumerate(gch):
                nc.tensor.matmul(
                    s_ps[:, sl * QTILE:(sl + 1) * QTILE],
                    lhsT=kstk[:, cc * CHUNK:(cc + 1) * CHUNK],
                    rhs=qT[:, j * QTILE:(j + 1) * QTILE],
                    start=True, stop=True,
                )
            pt = ptpool.tile([128, n * QTILE], bf16, tag="pt",
                             name=f"pt{j}_{gch[0]}")
            nc.scalar.activation(
                pt, s_ps, mybir.ActivationFunctionType.Exp, scale=SCALE
            )
            if kind in ("own_diag01", "mixed"):
                nc.vector.tensor_mul(pt[:, 0:2 * QTILE], pt[:, 0:2 * QTILE],
                                     masks)
            if kind == "own_diag":
                nc.vector.tensor_mul(pt[:, 2 * QTILE:], pt[:, 2 * QTILE:],
                                     masks)
            if kind in ("oth_tail", "mixed"):
                nc.vector.tensor_scalar_mul(
                    pt[:, 2 * QTILE:], pt[:, 2 * QTILE:], scal_sb[:, j:j + 1]
                )
            if kind == "oth01":
                nc.vector.tensor_scalar_mul(pt, pt, scal_sb[:, j:j + 1])
            pt_tiles[(j, gch[0])] = pt

        # PSUM discipline: only ONE accumulation chain may be open per psum
        # bank at a time (an interleaved start=True wipes the open chain).
        # So each PV group is a CLOSED 4-matmul chain into a small rotating
        # psum tile; cross-group accumulation happens in SBUF on vector.
        o_acc = kqpool.tile([D + 1, N_SLOTS, QTILE], fp32)
        pv_done = {j: 0 for j in range(N_SLOTS)}

        def pv(j, gch):
            """PV for one group of slot j (closed psum chain + SBUF accum);
            DMA out when the slot completes."""
            pt = pt_tiles.pop((j, gch[0]))
            # full-bank tile (2KB/part): a start=True chain may clobber its
            # whole psum bank, so no two live tiles may share one.
            g_ps = psO.tile([D + 1, 2 * QTILE], fp32, tag="o",
                            name=f"ops{j}_{gch[0]}")[:, 0:QTILE]
            for sl, cc in enumerate(gch):
                mi = nc.tensor.matmul(
                    g_ps,
                    lhsT=vones[:, cc, :],
                    rhs=pt[:, sl * QTILE:(sl + 1) * QTILE],
                    start=(sl == 0), stop=(sl == len(gch) - 1),
                )
                # the strided partial-inner V-copy region is mistracked by
                # Tile's dep layer — enforce copy -> PV-read explicitly
                # (arg order: first WAITS ON second)
                add_dep_helper(mi.ins, vcopy_inst[cc].ins, sync=True,
                               reason="PV matmul waits on vones V cols")
            first = pv_done[j] == 0
            pv_done[j] += len(gch)
            if first:
                nc.vector.tensor_copy(o_acc[:, j, :], g_ps)
            else:
                nc.vector.tensor_add(o_acc[:, j, :], o_acc[:, j, :], g_ps)
            if pv_done[j] == NCH_OF_SLOT[j]:
                nc.gpsimd.dma_start(
                    out=out[:, j * QTILE:(j + 1) * QTILE], in_=o_acc[:, j, :]
                )

        # ---- emission order = scheduler priority (engines pop the
        # lowest-priority READY instruction; blocked work never stalls
        # an engine). Order: exp-chain-critical work per slab. ----
        proj(0)
        for j, gch, kind in GROUPS_BY_SLAB[0]:
            sexp(j, gch, kind)
        transp(0, 4)
        proj(1)
        g1 = GROUPS_BY_SLAB[1]
        sexp(*g1[0])
        pv(0, [0, 1])           # (0,0a)
        sexp(*g1[1])
        pv(1, [0, 1, 2, 3])     # (1,0)
        sexp(*g1[2])
        transp(4, 4)
        proj(2)
        g2 = GROUPS_BY_SLAB[2]
        sexp(*g2[0])
        pv(2, [0, 1, 2, 3])     # (2,0)
        sexp(*g2[1])
        pv(3, [0, 1, 2, 3])     # (3,0)
        sexp(*g2[2])
        pv(3, [4, 5, 6, 7])     # (3,2)
        sexp(*g2[3])
        transp(8, 4)
        pv(0, [8, 9])           # (0,0b) -> finalizes slot 0
        proj(3)
        g3 = GROUPS_BY_SLAB[3]
        pv(1, [8, 9, 10, 11])   # (1,1) -> finalizes slot 1
        sexp(*g3[0])
        pv(2, [8, 9, 10, 11])   # (2,1)
        transp(12, 4)
        sexp(*g3[1])
        pv(3, [8, 9, 10, 11])   # (3,1)
        pv(2, [4, 5, 12, 13])   # (2,2) -> finalizes slot 2
        pv(3, [12, 13, 14, 15])  # (3,3) -> finalizes slot 3

        # chained ACT warmers, emitted LAST = lowest scheduler priority:
        # keep the scalar engine's pipeline warm until the exp chain
        # starts (each depends on the previous; SBUF only).
        wsc = consts.tile([128, 512], bf16)
        nc.vector.memset(wsc, 0.0)
        for w_ in range(10):
            nc.scalar.activation(
                wsc, wsc, mybir.ActivationFunctionType.Exp, scale=0.0
            )

    nc.compile()
    return nc


_NC_CACHE = None


def _get_nc():
    global _NC_CACHE
    if _NC_CACHE is None:
        _NC_CACHE = _build_graph()
    return _NC_CACHE


def _perm_tiles(r):
    """permuted 256-col tile order: own tiles (2j+r) first, then others."""
    own = [2 * j + r for j in range(N_SLOTS)]
    oth = [2 * j + (1 - r) for j in range(N_SLOTS)]
    return own + oth


def _host_prep(x, W_Q, W_K):
    in_maps = []
    w = np.concatenate([W_K.T, W_Q.T], axis=1).astype(BF16)  # [1024, 128]
    wkq = np.ascontiguousarray(w.reshape(8, 128, 2 * D).transpose(1, 0, 2))
    for i in range(N_CORES):
        b, r = i % B, i // B
        perm = _perm_tiles(r)
        xt = x[b].T.astype(BF16)  # [1024, 2048]
        cols = np.concatenate(
            [np.arange(QTILE * p, QTILE * p + QTILE) for p in perm]
        )
        xkt = xt[:, cols]  # [1024, 2048] permuted
        # slab-major: [slab, part, cchunk, 512]
        x4 = np.ascontiguousarray(
            xkt.reshape(8, 128, 4, 512).transpose(2, 1, 0, 3)
        )
        sc = np.full((CHUNK, N_SLOTS), float(r), dtype=np.float32)
        in_maps.append({"x4": x4, "wkq": wkq, "scal": sc})
    return in_maps


def _ensure_ntff_hook():
    """Install the antenv.axon_hooks shim so trace=True works under axon."""
    import types

    try:
        from antenv.axon_hooks import get_axon_ntff_profile_hook  # noqa: F401

        return
    except ImportError:
        pass
    import antenv

    mod = types.ModuleType("antenv.axon_hooks")
    mod._hook = None

    def set_axon_ntff_profile_hook(h):
        mod._hook = h

    def get_axon_ntff_profile_hook():
        return mod._hook

    mod.set_axon_ntff_profile_hook = set_axon_ntff_profile_hook
    mod.get_axon_ntff_profile_hook = get_axon_ntff_profile_hook
    sys.modules["antenv.axon_hooks"] = mod
    antenv.axon_hooks = mod
    try:
        from trn_agent_boot.trn_boot import _ntff_profile_via_ctypes

        hook = _ntff_profile_via_ctypes("/opt/axon/libaxon_pjrt.so")
        if hook is not None:
            set_axon_ntff_profile_hook(hook)
    except Exception as e:  # degrade to no tracing
        print(f"ntff hook install failed: {e}")


def kernel(x, W_Q, W_K, W_V=None, **_unused):
    global LAST_RESULTS
    if TRACE:
        _ensure_ntff_hook()
    x = np.asarray(x, dtype=np.float32)
    W_Q = np.asarray(W_Q, dtype=np.float32)
    W_K = np.asarray(W_K, dtype=np.float32)

    from concourse.bass_utils import run_bass_kernel_spmd

    nc = _get_nc()
    in_maps = _host_prep(x, W_Q, W_K)
    res = run_bass_kernel_spmd(
        nc,
        in_maps,
        core_ids=list(range(N_CORES)),
        trace=TRACE,
        trace_cores=TRACE_CORES,
    )
    LAST_RESULTS = res

    y = np.empty((B, T, D), dtype=np.float32)
    for i in range(N_CORES):
        b, r = i % B, i // B
        ot = res.results[i]["out"]  # [65, 1024]
        o = ot[0:D, :] / ot[D:D + 1, :]
        for j in range(N_SLOTS):
            t0 = QTILE * (2 * j + r)
            y[b, t0:t0 + QTILE, :] = o[:, j * QTILE:(j + 1) * QTILE].T
    return y
